# revision 14
# baseline (speedup 1.0000x reference)
import numpy as np

EPS = 1e-5

# nn_AssociativeAttention: B=2, L=2048, D=256, H=8, h=32.
# 8 cores x 2 (b,h) lanes each (same b, 2 heads per core).
# Per lane the outer-product cumsum is chunked causal linear attention:
#   ctxt_l = w_l * (sum_{i<=l} g_i (q_l.vn_i) kn_i + q_l @ S_prev)
# with S updated per 128-chunk; online-softmax & gate scans run on-device
# via tensor_tensor_scan in a [32,128] segment layout (p = lane*16+seg).
# Each core emits a partial out-projection [2048,256] for its two heads;
# host reduces over the 4 cores per batch and adds wo_b.

B, L, D, H, HD = 2, 2048, 256, 8, 32
N_CORES = 8
C = 128
NCH = L // C
SEG = 16


def _build_bass():
    import sys
    if '/opt/trn_rl_repo' not in sys.path:
        sys.path.insert(0, '/opt/trn_rl_repo')
    from contextlib import ExitStack
    import concourse.bacc as bacc
    import concourse.tile as tile
    from concourse import mybir

    f32 = mybir.dt.float32
    AF = mybir.ActivationFunctionType
    OP = mybir.AluOpType

    nc = bacc.Bacc()
    dt = nc.dram_tensor
    xT_d = dt("xT", [128, 2, L], f32, kind="ExternalInput")      # x_b^T regrouped
    wqkvT_d = dt("wqkvT", [128, 2, 192], f32, kind="ExternalInput")
    woT_d = dt("woT", [64, D], f32, kind="ExternalInput")
    bdwg_d = dt("bdwg", [64, 64], f32, kind="ExternalInput")
    bd2_d = dt("bd2", [64, 2], f32, kind="ExternalInput")
    bd64_d = dt("bd64", [2, 64], f32, kind="ExternalInput")
    lt32_d = dt("lt32", [32, 32], f32, kind="ExternalInput")
    ident_d = dt("ident", [128, 128], f32, kind="ExternalInput")
    mask_d = dt("mask", [128, 128], f32, kind="ExternalInput")
    qs32_d = dt("qs32", [32, 1], f32, kind="ExternalInput")
    vecs_d = dt("vecs", [64, 2], f32, kind="ExternalInput")      # a | b fold vecs
    gb_d = dt("gb", [32, 1], f32, kind="ExternalInput")
    y_d = dt("y", [L, D], f32, kind="ExternalOutput")

    with tile.TileContext(nc) as tc:
        with ExitStack() as ctx:
            const = ctx.enter_context(tc.tile_pool(name="const", bufs=1))
            big = ctx.enter_context(tc.tile_pool(name="big", bufs=1))
            work = ctx.enter_context(tc.tile_pool(name="work", bufs=2))
            outp = ctx.enter_context(tc.tile_pool(name="outp", bufs=3))
            pmm = ctx.enter_context(tc.tile_pool(name="pmm", bufs=2, space="PSUM"))
            psc = ctx.enter_context(tc.tile_pool(name="psc", bufs=1, space="PSUM"))
            pmo = ctx.enter_context(tc.tile_pool(name="pmo", bufs=5, space="PSUM"))

            def cdma(tile_ap, dram_ap):
                nc.sync.dma_start(tile_ap, dram_ap)

            bdwg = const.tile([64, 64], f32)
            cdma(bdwg[:], bdwg_d[:, :])
            bd2 = const.tile([64, 2], f32)
            cdma(bd2[:], bd2_d[:, :])
            bd64 = const.tile([2, 64], f32)
            cdma(bd64[:], bd64_d[:, :])
            lt32 = const.tile([32, 32], f32)
            cdma(lt32[:], lt32_d[:, :])
            ident = const.tile([128, 128], f32)
            cdma(ident[:], ident_d[:, :])
            mask = const.tile([128, 128], f32)
            cdma(mask[:], mask_d[:, :])
            qs32 = const.tile([32, 1], f32)
            cdma(qs32[:], qs32_d[:, :])
            vecs = const.tile([64, 2], f32)
            cdma(vecs[:], vecs_d[:, :])
            gb = const.tile([32, 1], f32)
            cdma(gb[:], gb_d[:, :])
            woT = const.tile([64, D], f32)
            cdma(woT[:], woT_d[:, :])
            wqkvT = const.tile([128, 2, 192], f32)
            cdma(wqkvT[:], wqkvT_d[:, :, :])
            xT = big.tile([128, 2, L], f32)
            for blk in range(4):
                bsl = slice(blk*512, (blk+1)*512)
                cdma(xT[:, :, bsl], xT_d[:, :, bsl])
            z32 = const.tile([32, 128], f32)
            nc.vector.memset(z32[:], 0.0)
            z1 = const.tile([1, 32], f32)
            nc.vector.memset(z1[:], 0.0)

            # ---- projections: qT/kT/vT [64, 2048] ----
            qT = big.tile([64, L], f32)
            kT = big.tile([64, L], f32)
            vT = big.tile([64, L], f32)
            for j, dst in ((1, kT), (2, vT), (0, qT)):
                for blk in range(4):
                    ps = pmm.tile([64, 512], f32, tag="mm")
                    for f in range(2):
                        nc.tensor.matmul(ps[:], wqkvT[:, f, j*64:(j+1)*64],
                                         xT[:, f, blk*512:(blk+1)*512],
                                         start=(f == 0), stop=(f == 1))
                    nc.vector.tensor_copy(dst[:, blk*512:(blk+1)*512], ps[:])

            # ---- sim + |k|^2 stacked colsum -> [4,2048] -> [32,128] ----
            stk = big.tile([128, L], f32)
            nc.vector.tensor_mul(stk[0:64, :], qT[:], kT[:])
            nc.vector.tensor_mul(stk[64:128, :], kT[:], kT[:])
            bd128 = const.tile([128, 4], f32)
            nc.vector.memset(bd128[:], 0.0)
            nc.vector.tensor_copy(bd128[0:64, 0:2], bd2[:])
            nc.vector.tensor_copy(bd128[64:128, 2:4], bd2[:])
            simnk = big.tile([4, L], f32)
            for blk in range(4):
                ps = psc.tile([4, 512], f32, tag="cs")
                nc.tensor.matmul(ps[:], bd128[:], stk[:, blk*512:(blk+1)*512],
                                 start=True, stop=True)
                nc.vector.tensor_copy(simnk[:, blk*512:(blk+1)*512], ps[:])
            sim32 = work.tile([32, 128], f32, tag="s32a")
            nc.gpsimd.dma_start(sim32[:],
                                simnk[0:2, :].rearrange("p (s t) -> p s t", s=SEG))
            nk32 = work.tile([32, 128], f32, tag="s32b")
            nc.gpsimd.dma_start(nk32[:],
                                simnk[2:4, :].rearrange("p (s t) -> p s t", s=SEG))

            # ---- |v|^2 colsum (reuse stk rows 64:) ----
            stk2 = big.tile([128, L], f32)
            nc.gpsimd.memset(stk2[0:64, :], 0.0)
            nc.vector.tensor_mul(stk2[64:128, :], vT[:], vT[:])
            simnv = big.tile([4, L], f32)
            for blk in range(4):
                ps = psc.tile([4, 512], f32, tag="cs")
                nc.tensor.matmul(ps[:], bd128[:], stk2[:, blk*512:(blk+1)*512],
                                 start=True, stop=True)
                nc.vector.tensor_copy(simnv[:, blk*512:(blk+1)*512], ps[:])
            nv32 = work.tile([32, 128], f32, tag="s32c")
            nc.gpsimd.dma_start(nv32[:],
                                simnv[2:4, :].rearrange("p (s t) -> p s t", s=SEG))

            # ---- inverse norms -> [2,2048] ----
            def inv_sqrt32(src32, tg):
                t = work.tile([32, 128], f32, tag=tg + "t")
                nc.scalar.sqrt(t[:], src32[:])
                o = work.tile([32, 128], f32, tag=tg + "o")
                nc.vector.reciprocal(o[:], t[:])
                return o

            invk32 = inv_sqrt32(nk32, "ik")
            invk2 = big.tile([2, L], f32)
            nc.gpsimd.dma_start(invk2.rearrange("p (s t) -> p s t", s=SEG), invk32[:])
            invv32 = inv_sqrt32(nv32, "iv")
            invv2 = big.tile([2, L], f32)
            nc.gpsimd.dma_start(invv2.rearrange("p (s t) -> p s t", s=SEG), invv32[:])

            # kn = kT * bvec * invk64 ; vn = vT * a * invv64
            kn = big.tile([64, L], f32)
            vn = big.tile([64, L], f32)
            for blk in range(4):
                bsl = slice(blk*512, (blk+1)*512)
                ivk = pmm.tile([64, 512], f32, tag="mm")
                nc.tensor.matmul(ivk[:], bd64[:], invk2[:, bsl], start=True, stop=True)
                nc.vector.scalar_tensor_tensor(kn[:, bsl], kT[:, bsl], vecs[:, 1:2],
                                               ivk[:], OP.mult, OP.mult)
                ivv = pmm.tile([64, 512], f32, tag="mm")
                nc.tensor.matmul(ivv[:], bd64[:], invv2[:, bsl], start=True, stop=True)
                nc.vector.scalar_tensor_tensor(vn[:, bsl], vT[:, bsl], vecs[:, 0:1],
                                               ivv[:], OP.mult, OP.mult)

            # ---- gate logits ----
            p2 = big.tile([64, L], f32)
            glog2 = big.tile([2, L], f32)
            for blk in range(4):
                bsl = slice(blk*512, (blk+1)*512)
                tps = pmm.tile([64, 512], f32, tag="mm")
                nc.tensor.matmul(tps[:], bdwg[:], kn[:, bsl], start=True, stop=True)
                nc.vector.tensor_mul(p2[:, bsl], vn[:, bsl], tps[:])
                gps = psc.tile([4, 512], f32, tag="cs")
                nc.tensor.matmul(gps[0:2, :], bd2[:], p2[:, bsl], start=True, stop=True)
                nc.vector.tensor_copy(glog2[:, bsl], gps[0:2, :])
            glog32 = work.tile([32, 128], f32, tag="s32d")
            nc.gpsimd.dma_start(glog32[:], glog2.rearrange("p (s t) -> p s t", s=SEG))

            # ---- scalar chain in [32,128] ----
            grelu = work.tile([32, 128], f32, tag="sc1")
            nc.scalar.activation(grelu[:], glog32[:], AF.Relu, bias=gb[:, 0:1], scale=1.0)
            g32 = work.tile([32, 128], f32, tag="sc2")
            nc.scalar.square(g32[:], grelu[:])
            nc.vector.tensor_scalar_add(g32[:], g32[:], EPS)
            sims = work.tile([32, 128], f32, tag="sc3")
            nc.scalar.activation(sims[:], sim32[:], AF.Copy, bias=0.0, scale=qs32[:, 0:1])
            # cummax
            mloc = work.tile([32, 128], f32, tag="sc4")
            nc.vector.tensor_tensor_scan(mloc[:], sims[:], z32[:], -1e30, OP.max, OP.add)
            endsT = psc.tile([1, 32], f32, tag="cs")
            nc.tensor.transpose(endsT[:], mloc[:, 127:128], ident[0:32, 0:32])
            endsT_sb = work.tile([1, 32], f32, tag="tp2")
            nc.vector.tensor_copy(endsT_sb[:], endsT[:])
            offm_row = work.tile([1, 32], f32, tag="tp3")
            nc.vector.memset(offm_row[:], -1e30)
            for lane in range(2):
                nc.vector.tensor_tensor_scan(
                    offm_row[:, lane*16+1:(lane+1)*16],
                    endsT_sb[:, lane*16:(lane+1)*16-1],
                    z1[:, 0:15], -1e30, OP.max, OP.add)
            offm = psc.tile([32, 1], f32, tag="cs")
            nc.tensor.transpose(offm[:], offm_row[:], ident[0:1, 0:1])
            offm_sb = work.tile([32, 1], f32, tag="tp5")
            nc.vector.tensor_copy(offm_sb[:], offm[:])
            m32 = work.tile([32, 128], f32, tag="sc5")
            nc.vector.scalar_tensor_tensor(m32[:], mloc[:], offm_sb[:, 0:1], mloc[:],
                                           OP.max, OP.max)
            # e, c, r, s, sw, coef
            e32 = work.tile([32, 128], f32, tag="sc6")
            nc.scalar.activation(e32[:], sims[:], AF.Exp, bias=0.0, scale=1.0)
            cloc = work.tile([32, 128], f32, tag="sc7")
            nc.vector.tensor_tensor_scan(cloc[:], e32[:], z32[:], 0.0, OP.add, OP.add)
            coff = psc.tile([32, 1], f32, tag="cs")
            nc.tensor.matmul(coff[:], lt32[:], cloc[:, 127:128], start=True, stop=True)
            coff_sb = work.tile([32, 1], f32, tag="tp7")
            nc.vector.tensor_copy(coff_sb[:], coff[:])
            c32 = work.tile([32, 128], f32, tag="sc8")
            nc.scalar.activation(c32[:], cloc[:], AF.Identity, bias=coff_sb[:, 0:1],
                                 scale=1.0)
            r32 = work.tile([32, 128], f32, tag="sc9")
            nc.scalar.activation(r32[:], m32[:], AF.Exp, bias=0.0, scale=-1.0)
            s32 = work.tile([32, 128], f32, tag="sc10")
            nc.vector.tensor_mul(s32[:], c32[:], r32[:])
            nc.vector.tensor_scalar_add(s32[:], s32[:], EPS)
            rs32 = work.tile([32, 128], f32, tag="sc11")
            nc.vector.reciprocal(rs32[:], s32[:])
            sw32 = work.tile([32, 128], f32, tag="sc12")
            nc.vector.tensor_mul(sw32[:], e32[:], r32[:])
            nc.vector.tensor_mul(sw32[:], sw32[:], rs32[:])
            coef = work.tile([32, 128], f32, tag="sc13")
            nc.scalar.activation(coef[:], sw32[:], AF.Silu, bias=0.0, scale=1.0)
            nc.vector.tensor_scalar_add(coef[:], coef[:], 1.0)
            # g_scan -> w = coef/(gs+EPS)
            gloc = work.tile([32, 128], f32, tag="sc14")
            nc.vector.tensor_tensor_scan(gloc[:], g32[:], z32[:], 0.0, OP.add, OP.add)
            gsoff = psc.tile([32, 1], f32, tag="cs")
            nc.tensor.matmul(gsoff[:], lt32[:], gloc[:, 127:128], start=True, stop=True)
            gsoff_sb = work.tile([32, 1], f32, tag="tp9")
            nc.vector.tensor_copy(gsoff_sb[:], gsoff[:])
            nc.vector.tensor_scalar_add(gsoff_sb[:], gsoff_sb[:], EPS)
            gs32 = work.tile([32, 128], f32, tag="sc15")
            nc.scalar.activation(gs32[:], gloc[:], AF.Identity, bias=gsoff_sb[:, 0:1],
                                 scale=1.0)
            rg32 = work.tile([32, 128], f32, tag="sc16")
            nc.vector.reciprocal(rg32[:], gs32[:])
            w32 = work.tile([32, 128], f32, tag="sc17")
            nc.vector.tensor_mul(w32[:], coef[:], rg32[:])
            g2 = big.tile([2, L], f32)
            nc.gpsimd.dma_start(g2.rearrange("p (s t) -> p s t", s=SEG), g32[:])

            # vng = vn * g64 ; w stays in [32,128] -> transpose to R layout
            vng = big.tile([64, L], f32)
            for blk in range(4):
                bsl = slice(blk*512, (blk+1)*512)
                gps2 = pmm.tile([64, 512], f32, tag="mm")
                nc.tensor.matmul(gps2[:], bd64[:], g2[:, bsl], start=True, stop=True)
                nc.vector.tensor_mul(vng[:, bsl], vn[:, bsl], gps2[:])
            wR_ps = psc.tile([128, 32], f32, tag="cs")
            nc.tensor.transpose(wR_ps[:], w32[:], ident[0:32, 0:32])
            wR = work.tile([128, 32], f32, tag="wR")
            nc.vector.tensor_copy(wR[:], wR_ps[:])

            # ---- chunked causal linear attention + partial out-proj ----
            Sboth = const.tile([64, 32], f32, tag="Sboth")
            nc.vector.memset(Sboth[:], 0.0)
            for cidx in range(NCH):
                sl = slice(cidx*C, (cidx+1)*C)
                knT_ps = pmo.tile([128, 64], f32, tag="o")
                nc.tensor.transpose(knT_ps[:], kn[:, sl], ident[0:64, 0:64])
                knR = work.tile([128, 64], f32, tag="knR")
                nc.vector.tensor_copy(knR[:], knT_ps[:])
                vngT_ps = pmo.tile([128, 64], f32, tag="o")
                nc.tensor.transpose(vngT_ps[:], vng[:, sl], ident[0:64, 0:64])
                vngR = work.tile([128, 64], f32, tag="vngR")
                nc.vector.tensor_copy(vngR[:], vngT_ps[:])
                ctxt_sb = outp.tile([128, 64], f32, tag="ctxt")
                for lane in range(2):
                    lsl = slice(32*lane, 32*lane+32)
                    at_ps = pmo.tile([128, 128], f32, tag="o")
                    nc.tensor.matmul(at_ps[:], vng[lsl, sl], qT[lsl, sl],
                                     start=True, stop=True)
                    atm = work.tile([128, 128], f32, tag="atm")
                    nc.vector.tensor_mul(atm[:], at_ps[:], mask[:])
                    ct_ps = pmo.tile([128, 32], f32, tag="o")
                    nc.tensor.matmul(ct_ps[:], qT[lsl, sl], Sboth[lsl, :],
                                     start=True, stop=False)
                    nc.tensor.matmul(ct_ps[:], atm[:], knR[:, lsl],
                                     start=False, stop=True)
                    nc.vector.tensor_scalar(
                        ctxt_sb[:, lsl], ct_ps[:],
                        wR[:, lane*16+cidx:lane*16+cidx+1], None, OP.mult)
                    dS_ps = pmo.tile([32, 32], f32, tag="o")
                    nc.tensor.matmul(dS_ps[:], vngR[:, lsl], knR[:, lsl],
                                     start=True, stop=True)
                    nc.vector.tensor_add(Sboth[lsl, :], Sboth[lsl, :], dS_ps[:])
                ctxtT_ps = pmo.tile([64, 128], f32, tag="o")
                nc.tensor.transpose(ctxtT_ps[:], ctxt_sb[:], ident[:, :])
                ctxtT_sb = outp.tile([64, 128], f32, tag="ctxtTs")
                nc.vector.tensor_copy(ctxtT_sb[:], ctxtT_ps[:])
                y_ps = pmo.tile([128, 256], f32, tag="o")
                nc.tensor.matmul(y_ps[:], ctxtT_sb[:], woT[:], start=True, stop=True)
                y_sb = outp.tile([128, 256], f32, tag="ysb")
                nc.scalar.copy(y_sb[:], y_ps[:])
                nc.sync.dma_start(y_d[sl, :], y_sb[:])

    global _LAST_TC_SPAN
    try:
        _LAST_TC_SPAN = max(e[2] for e in tc._perfetto_entries)
    except Exception:
        _LAST_TC_SPAN = 0
    nc.compile()
    return nc


_NC_CACHE = None
_LAST_IN_MAPS = None
_LAST_TC_SPAN = 0


def _get_nc():
    global _NC_CACHE
    if _NC_CACHE is None:
        _NC_CACHE = _build_bass()
    return _NC_CACHE


def kernel(**inputs):
    import sys
    if '/opt/trn_rl_repo' not in sys.path:
        sys.path.insert(0, '/opt/trn_rl_repo')
    from concourse.bass_utils import run_bass_kernel_spmd

    inp = {k: np.asarray(v) for k, v in inputs.items()}
    x = inp['x'].astype(np.float32)
    wq, wk, wv, wo = (inp[n].astype(np.float32) for n in ('wq_w', 'wk_w', 'wv_w', 'wo_w'))
    wg = inp['wg_w'].astype(np.float32).reshape(HD, HD)
    gbv = float(inp['wg_b'][0])
    kvs = inp['kv_norm_scale'].astype(np.float32)[0, :, 0]
    qks = inp['qk_norm_scale'].astype(np.float32)[0, :, 0]

    nc = _get_nc()

    bd2 = np.zeros((64, 2), np.float32)
    bd2[0:32, 0] = 1.0
    bd2[32:64, 1] = 1.0
    bd64 = np.zeros((2, 64), np.float32)
    bd64[0, 0:32] = 1.0
    bd64[1, 32:64] = 1.0
    lt32 = np.zeros((32, 32), np.float32)
    for p in range(32):
        for s in range(32):
            if p // 16 == s // 16 and s % 16 < p % 16:
                lt32[s, p] = 1.0
    ident = np.eye(128, dtype=np.float32)
    maskc = (np.arange(128)[:, None] <= np.arange(128)[None, :]).astype(np.float32)

    in_maps = []
    for core in range(N_CORES):
        b = core // 4
        heads = (2 * (core % 4), 2 * (core % 4) + 1)
        xT = np.ascontiguousarray(
            x[b].T.reshape(2, 128, L).transpose(1, 0, 2))       # [128,2,L]
        wqkvT = np.empty((D, 192), np.float32)
        for j, wmat in enumerate((wq, wk, wv)):
            for li, hh in enumerate(heads):
                wqkvT[:, j*64+li*32:j*64+(li+1)*32] = wmat[hh*HD:(hh+1)*HD, :].T
        wqkvT = np.ascontiguousarray(wqkvT.reshape(2, 128, 192).transpose(1, 0, 2))
        woT = np.empty((64, D), np.float32)
        for li, hh in enumerate(heads):
            woT[li*32:(li+1)*32, :] = wo[:, hh*HD:(hh+1)*HD].T
        bdwg = np.zeros((64, 64), np.float32)
        vecs = np.ones((64, 2), np.float32)
        for li, hh in enumerate(heads):
            sc = kvs[hh]
            a = sc[:, 0].copy()
            bvec = sc[0, :] / sc[0, 0]
            Wg_h = wg * sc / np.outer(a, bvec)
            bdwg[li*32:(li+1)*32, li*32:(li+1)*32] = Wg_h.T
            vecs[li*32:(li+1)*32, 0] = a
            vecs[li*32:(li+1)*32, 1] = bvec
        qs32 = np.empty((32, 1), np.float32)
        qs32[0:16] = qks[heads[0]]
        qs32[16:32] = qks[heads[1]]
        gb32 = np.full((32, 1), gbv, np.float32)
        in_maps.append({
            "xT": xT, "wqkvT": wqkvT, "woT": woT, "bdwg": bdwg,
            "bd2": bd2, "bd64": bd64, "lt32": lt32, "ident": ident,
            "mask": maskc, "qs32": qs32, "vecs": vecs, "gb": gb32,
        })

    global _LAST_IN_MAPS
    _LAST_IN_MAPS = in_maps
    res = run_bass_kernel_spmd(nc, in_maps, core_ids=list(range(N_CORES)))
    out = np.zeros((B, L, D), np.float32)
    for core in range(N_CORES):
        out[core // 4] += res.results[core]["y"]
    out += inp['wo_b'].astype(np.float32)[None, None, :]
    return out


# revision 15
# speedup vs baseline: 1.0135x; 1.0135x over previous
import numpy as np

EPS = 1e-5

# nn_AssociativeAttention: B=2, L=2048, D=256, H=8, h=32.
# 8 cores x 2 (b,h) lanes each (same b, 2 heads per core).
# Per lane the outer-product cumsum is chunked causal linear attention:
#   ctxt_l = w_l * (sum_{i<=l} g_i (q_l.vn_i) kn_i + q_l @ S_prev)
# with S updated per 128-chunk; online-softmax & gate scans run on-device
# via tensor_tensor_scan in a [32,128] segment layout (p = lane*16+seg).
# Each core emits a partial out-projection [2048,256] for its two heads;
# host reduces over the 4 cores per batch and adds wo_b.

B, L, D, H, HD = 2, 2048, 256, 8, 32
N_CORES = 8
C = 128
NCH = L // C
SEG = 16


def _build_bass():
    import sys
    if '/opt/trn_rl_repo' not in sys.path:
        sys.path.insert(0, '/opt/trn_rl_repo')
    from contextlib import ExitStack
    import concourse.bacc as bacc
    import concourse.tile as tile
    from concourse import mybir

    f32 = mybir.dt.float32
    AF = mybir.ActivationFunctionType
    OP = mybir.AluOpType

    nc = bacc.Bacc()
    dt = nc.dram_tensor
    xT_d = dt("xT", [128, 2, L], f32, kind="ExternalInput")      # x_b^T regrouped
    wqkvT_d = dt("wqkvT", [128, 2, 192], f32, kind="ExternalInput")
    woT_d = dt("woT", [64, D], f32, kind="ExternalInput")
    bdwg_d = dt("bdwg", [64, 64], f32, kind="ExternalInput")
    bd2_d = dt("bd2", [64, 2], f32, kind="ExternalInput")
    bd64_d = dt("bd64", [2, 64], f32, kind="ExternalInput")
    lt32_d = dt("lt32", [32, 32], f32, kind="ExternalInput")
    ident_d = dt("ident", [128, 128], f32, kind="ExternalInput")
    mask_d = dt("mask", [128, 128], f32, kind="ExternalInput")
    qs32_d = dt("qs32", [32, 1], f32, kind="ExternalInput")
    vecs_d = dt("vecs", [64, 2], f32, kind="ExternalInput")      # a | b fold vecs
    gb_d = dt("gb", [32, 1], f32, kind="ExternalInput")
    y_d = dt("y", [L, D], f32, kind="ExternalOutput")

    with tile.TileContext(nc) as tc:
        with ExitStack() as ctx:
            const = ctx.enter_context(tc.tile_pool(name="const", bufs=1))
            big = ctx.enter_context(tc.tile_pool(name="big", bufs=1))
            work = ctx.enter_context(tc.tile_pool(name="work", bufs=3))
            outp = ctx.enter_context(tc.tile_pool(name="outp", bufs=4))
            pmm = ctx.enter_context(tc.tile_pool(name="pmm", bufs=3, space="PSUM"))
            psc = ctx.enter_context(tc.tile_pool(name="psc", bufs=1, space="PSUM"))
            pmo = ctx.enter_context(tc.tile_pool(name="pmo", bufs=4, space="PSUM"))

            def cdma(tile_ap, dram_ap):
                nc.sync.dma_start(tile_ap, dram_ap)

            bdwg = const.tile([64, 64], f32)
            cdma(bdwg[:], bdwg_d[:, :])
            bd2 = const.tile([64, 2], f32)
            cdma(bd2[:], bd2_d[:, :])
            bd64 = const.tile([2, 64], f32)
            cdma(bd64[:], bd64_d[:, :])
            lt32 = const.tile([32, 32], f32)
            cdma(lt32[:], lt32_d[:, :])
            ident = const.tile([128, 128], f32)
            cdma(ident[:], ident_d[:, :])
            mask = const.tile([128, 128], f32)
            cdma(mask[:], mask_d[:, :])
            qs32 = const.tile([32, 1], f32)
            cdma(qs32[:], qs32_d[:, :])
            vecs = const.tile([64, 2], f32)
            cdma(vecs[:], vecs_d[:, :])
            gb = const.tile([32, 1], f32)
            cdma(gb[:], gb_d[:, :])
            woT = const.tile([64, D], f32)
            cdma(woT[:], woT_d[:, :])
            wqkvT = const.tile([128, 2, 192], f32)
            cdma(wqkvT[:], wqkvT_d[:, :, :])
            xT = big.tile([128, 2, L], f32)
            for blk in range(4):
                bsl = slice(blk*512, (blk+1)*512)
                cdma(xT[:, :, bsl], xT_d[:, :, bsl])
            z32 = const.tile([32, 128], f32)
            nc.vector.memset(z32[:], 0.0)
            z1 = const.tile([1, 32], f32)
            nc.vector.memset(z1[:], 0.0)

            # ---- projections: qT/kT/vT [64, 2048] ----
            qT = big.tile([64, L], f32)
            kT = big.tile([64, L], f32)
            vT = big.tile([64, L], f32)
            for j, dst in ((1, kT), (2, vT), (0, qT)):
                for blk in range(4):
                    ps = pmm.tile([64, 512], f32, tag="mm")
                    for f in range(2):
                        nc.tensor.matmul(ps[:], wqkvT[:, f, j*64:(j+1)*64],
                                         xT[:, f, blk*512:(blk+1)*512],
                                         start=(f == 0), stop=(f == 1))
                    nc.scalar.copy(dst[:, blk*512:(blk+1)*512], ps[:])

            # ---- sim + |k|^2 stacked colsum -> [4,2048] -> [32,128] ----
            stk = big.tile([128, L], f32)
            nc.vector.tensor_mul(stk[0:64, :], qT[:], kT[:])
            nc.vector.tensor_mul(stk[64:128, :], kT[:], kT[:])
            bd128 = const.tile([128, 4], f32)
            nc.vector.memset(bd128[:], 0.0)
            nc.vector.tensor_copy(bd128[0:64, 0:2], bd2[:])
            nc.vector.tensor_copy(bd128[64:128, 2:4], bd2[:])
            simnk = big.tile([4, L], f32)
            for blk in range(4):
                ps = psc.tile([4, 512], f32, tag="cs")
                nc.tensor.matmul(ps[:], bd128[:], stk[:, blk*512:(blk+1)*512],
                                 start=True, stop=True)
                nc.scalar.copy(simnk[:, blk*512:(blk+1)*512], ps[:])
            sim32 = work.tile([32, 128], f32, tag="s32a")
            nc.gpsimd.dma_start(sim32[:],
                                simnk[0:2, :].rearrange("p (s t) -> p s t", s=SEG))
            nk32 = work.tile([32, 128], f32, tag="s32b")
            nc.gpsimd.dma_start(nk32[:],
                                simnk[2:4, :].rearrange("p (s t) -> p s t", s=SEG))

            # ---- |v|^2 colsum (reuse stk rows 64:) ----
            stk2 = big.tile([128, L], f32)
            nc.gpsimd.memset(stk2[0:64, :], 0.0)
            nc.vector.tensor_mul(stk2[64:128, :], vT[:], vT[:])
            simnv = big.tile([4, L], f32)
            for blk in range(4):
                ps = psc.tile([4, 512], f32, tag="cs")
                nc.tensor.matmul(ps[:], bd128[:], stk2[:, blk*512:(blk+1)*512],
                                 start=True, stop=True)
                nc.scalar.copy(simnv[:, blk*512:(blk+1)*512], ps[:])
            nv32 = work.tile([32, 128], f32, tag="s32c")
            nc.gpsimd.dma_start(nv32[:],
                                simnv[2:4, :].rearrange("p (s t) -> p s t", s=SEG))

            # ---- inverse norms -> [2,2048] ----
            def inv_sqrt32(src32, tg):
                t = work.tile([32, 128], f32, tag=tg + "t")
                nc.scalar.sqrt(t[:], src32[:])
                o = work.tile([32, 128], f32, tag=tg + "o")
                nc.vector.reciprocal(o[:], t[:])
                return o

            invk32 = inv_sqrt32(nk32, "ik")
            invk2 = big.tile([2, L], f32)
            nc.gpsimd.dma_start(invk2.rearrange("p (s t) -> p s t", s=SEG), invk32[:])
            invv32 = inv_sqrt32(nv32, "iv")
            invv2 = big.tile([2, L], f32)
            nc.gpsimd.dma_start(invv2.rearrange("p (s t) -> p s t", s=SEG), invv32[:])

            # kn = kT * bvec * invk64 ; vn = vT * a * invv64
            kn = big.tile([64, L], f32)
            vn = big.tile([64, L], f32)
            for blk in range(4):
                bsl = slice(blk*512, (blk+1)*512)
                ivk = pmm.tile([64, 512], f32, tag="mm")
                nc.tensor.matmul(ivk[:], bd64[:], invk2[:, bsl], start=True, stop=True)
                nc.vector.scalar_tensor_tensor(kn[:, bsl], kT[:, bsl], vecs[:, 1:2],
                                               ivk[:], OP.mult, OP.mult)
                ivv = pmm.tile([64, 512], f32, tag="mm")
                nc.tensor.matmul(ivv[:], bd64[:], invv2[:, bsl], start=True, stop=True)
                nc.vector.scalar_tensor_tensor(vn[:, bsl], vT[:, bsl], vecs[:, 0:1],
                                               ivv[:], OP.mult, OP.mult)

            # ---- gate logits ----
            p2 = big.tile([64, L], f32)
            glog2 = big.tile([2, L], f32)
            for blk in range(4):
                bsl = slice(blk*512, (blk+1)*512)
                tps = pmm.tile([64, 512], f32, tag="mm")
                nc.tensor.matmul(tps[:], bdwg[:], kn[:, bsl], start=True, stop=True)
                nc.vector.tensor_mul(p2[:, bsl], vn[:, bsl], tps[:])
                gps = psc.tile([4, 512], f32, tag="cs")
                nc.tensor.matmul(gps[0:2, :], bd2[:], p2[:, bsl], start=True, stop=True)
                nc.scalar.copy(glog2[:, bsl], gps[0:2, :])
            glog32 = work.tile([32, 128], f32, tag="s32d")
            nc.gpsimd.dma_start(glog32[:], glog2.rearrange("p (s t) -> p s t", s=SEG))

            # ---- scalar chain in [32,128] ----
            grelu = work.tile([32, 128], f32, tag="sc1")
            nc.scalar.activation(grelu[:], glog32[:], AF.Relu, bias=gb[:, 0:1], scale=1.0)
            g32 = work.tile([32, 128], f32, tag="sc2")
            nc.scalar.square(g32[:], grelu[:])
            nc.vector.tensor_scalar_add(g32[:], g32[:], EPS)
            sims = work.tile([32, 128], f32, tag="sc3")
            nc.scalar.activation(sims[:], sim32[:], AF.Copy, bias=0.0, scale=qs32[:, 0:1])
            # cummax
            mloc = work.tile([32, 128], f32, tag="sc4")
            nc.vector.tensor_tensor_scan(mloc[:], sims[:], z32[:], -1e30, OP.max, OP.add)
            endsT = psc.tile([1, 32], f32, tag="cs")
            nc.tensor.transpose(endsT[:], mloc[:, 127:128], ident[0:32, 0:32])
            endsT_sb = work.tile([1, 32], f32, tag="tp2")
            nc.vector.tensor_copy(endsT_sb[:], endsT[:])
            offm_row = work.tile([1, 32], f32, tag="tp3")
            nc.vector.memset(offm_row[:], -1e30)
            for lane in range(2):
                nc.vector.tensor_tensor_scan(
                    offm_row[:, lane*16+1:(lane+1)*16],
                    endsT_sb[:, lane*16:(lane+1)*16-1],
                    z1[:, 0:15], -1e30, OP.max, OP.add)
            offm = psc.tile([32, 1], f32, tag="cs")
            nc.tensor.transpose(offm[:], offm_row[:], ident[0:1, 0:1])
            offm_sb = work.tile([32, 1], f32, tag="tp5")
            nc.vector.tensor_copy(offm_sb[:], offm[:])
            m32 = work.tile([32, 128], f32, tag="sc5")
            nc.vector.scalar_tensor_tensor(m32[:], mloc[:], offm_sb[:, 0:1], mloc[:],
                                           OP.max, OP.max)
            # e, c, r, s, sw, coef
            e32 = work.tile([32, 128], f32, tag="sc6")
            nc.scalar.activation(e32[:], sims[:], AF.Exp, bias=0.0, scale=1.0)
            cloc = work.tile([32, 128], f32, tag="sc7")
            nc.vector.tensor_tensor_scan(cloc[:], e32[:], z32[:], 0.0, OP.add, OP.add)
            coff = psc.tile([32, 1], f32, tag="cs")
            nc.tensor.matmul(coff[:], lt32[:], cloc[:, 127:128], start=True, stop=True)
            coff_sb = work.tile([32, 1], f32, tag="tp7")
            nc.vector.tensor_copy(coff_sb[:], coff[:])
            c32 = work.tile([32, 128], f32, tag="sc8")
            nc.scalar.activation(c32[:], cloc[:], AF.Identity, bias=coff_sb[:, 0:1],
                                 scale=1.0)
            r32 = work.tile([32, 128], f32, tag="sc9")
            nc.scalar.activation(r32[:], m32[:], AF.Exp, bias=0.0, scale=-1.0)
            s32 = work.tile([32, 128], f32, tag="sc10")
            nc.vector.tensor_mul(s32[:], c32[:], r32[:])
            nc.vector.tensor_scalar_add(s32[:], s32[:], EPS)
            rs32 = work.tile([32, 128], f32, tag="sc11")
            nc.vector.reciprocal(rs32[:], s32[:])
            sw32 = work.tile([32, 128], f32, tag="sc12")
            nc.vector.tensor_mul(sw32[:], e32[:], r32[:])
            nc.vector.tensor_mul(sw32[:], sw32[:], rs32[:])
            coef = work.tile([32, 128], f32, tag="sc13")
            nc.scalar.activation(coef[:], sw32[:], AF.Silu, bias=0.0, scale=1.0)
            nc.vector.tensor_scalar_add(coef[:], coef[:], 1.0)
            # g_scan -> w = coef/(gs+EPS)
            gloc = work.tile([32, 128], f32, tag="sc14")
            nc.vector.tensor_tensor_scan(gloc[:], g32[:], z32[:], 0.0, OP.add, OP.add)
            gsoff = psc.tile([32, 1], f32, tag="cs")
            nc.tensor.matmul(gsoff[:], lt32[:], gloc[:, 127:128], start=True, stop=True)
            gsoff_sb = work.tile([32, 1], f32, tag="tp9")
            nc.vector.tensor_copy(gsoff_sb[:], gsoff[:])
            nc.vector.tensor_scalar_add(gsoff_sb[:], gsoff_sb[:], EPS)
            gs32 = work.tile([32, 128], f32, tag="sc15")
            nc.scalar.activation(gs32[:], gloc[:], AF.Identity, bias=gsoff_sb[:, 0:1],
                                 scale=1.0)
            rg32 = work.tile([32, 128], f32, tag="sc16")
            nc.vector.reciprocal(rg32[:], gs32[:])
            w32 = work.tile([32, 128], f32, tag="sc17")
            nc.vector.tensor_mul(w32[:], coef[:], rg32[:])
            g2 = big.tile([2, L], f32)
            nc.gpsimd.dma_start(g2.rearrange("p (s t) -> p s t", s=SEG), g32[:])

            # vng = vn * g64 ; w stays in [32,128] -> transpose to R layout
            vng = big.tile([64, L], f32)
            for blk in range(4):
                bsl = slice(blk*512, (blk+1)*512)
                gps2 = pmm.tile([64, 512], f32, tag="mm")
                nc.tensor.matmul(gps2[:], bd64[:], g2[:, bsl], start=True, stop=True)
                nc.vector.tensor_mul(vng[:, bsl], vn[:, bsl], gps2[:])
            wR_ps = psc.tile([128, 32], f32, tag="cs")
            nc.tensor.transpose(wR_ps[:], w32[:], ident[0:32, 0:32])
            wR = work.tile([128, 32], f32, tag="wR")
            nc.vector.tensor_copy(wR[:], wR_ps[:])

            # ---- chunked causal linear attention + partial out-proj ----
            Sboth = const.tile([64, 32], f32, tag="Sboth")
            nc.vector.memset(Sboth[:], 0.0)
            for cidx in range(NCH):
                sl = slice(cidx*C, (cidx+1)*C)
                knT_ps = pmo.tile([128, 64], f32, tag="o")
                nc.tensor.transpose(knT_ps[:], kn[:, sl], ident[0:64, 0:64])
                knR = work.tile([128, 64], f32, tag="knR")
                nc.vector.tensor_copy(knR[:], knT_ps[:])
                vngT_ps = pmo.tile([128, 64], f32, tag="o")
                nc.tensor.transpose(vngT_ps[:], vng[:, sl], ident[0:64, 0:64])
                vngR = work.tile([128, 64], f32, tag="vngR")
                nc.vector.tensor_copy(vngR[:], vngT_ps[:])
                ctxt_sb = outp.tile([128, 64], f32, tag="ctxt")
                for lane in range(2):
                    lsl = slice(32*lane, 32*lane+32)
                    at_ps = pmo.tile([128, 128], f32, tag="o")
                    nc.tensor.matmul(at_ps[:], vng[lsl, sl], qT[lsl, sl],
                                     start=True, stop=True)
                    atm = work.tile([128, 128], f32, tag="atm")
                    nc.vector.tensor_mul(atm[:], at_ps[:], mask[:])
                    ct_ps = pmo.tile([128, 32], f32, tag="o")
                    nc.tensor.matmul(ct_ps[:], qT[lsl, sl], Sboth[lsl, :],
                                     start=True, stop=False)
                    nc.tensor.matmul(ct_ps[:], atm[:], knR[:, lsl],
                                     start=False, stop=True)
                    nc.vector.tensor_scalar(
                        ctxt_sb[:, lsl], ct_ps[:],
                        wR[:, lane*16+cidx:lane*16+cidx+1], None, OP.mult)
                    dS_ps = pmo.tile([32, 32], f32, tag="o")
                    nc.tensor.matmul(dS_ps[:], vngR[:, lsl], knR[:, lsl],
                                     start=True, stop=True)
                    nc.vector.tensor_add(Sboth[lsl, :], Sboth[lsl, :], dS_ps[:])
                ctxtT_ps = pmo.tile([64, 128], f32, tag="o")
                nc.tensor.transpose(ctxtT_ps[:], ctxt_sb[:], ident[:, :])
                ctxtT_sb = outp.tile([64, 128], f32, tag="ctxtTs")
                nc.vector.tensor_copy(ctxtT_sb[:], ctxtT_ps[:])
                y_ps = pmo.tile([128, 256], f32, tag="o")
                nc.tensor.matmul(y_ps[:], ctxtT_sb[:], woT[:], start=True, stop=True)
                y_sb = outp.tile([128, 256], f32, tag="ysb")
                nc.scalar.copy(y_sb[:], y_ps[:])
                nc.sync.dma_start(y_d[sl, :], y_sb[:])

    global _LAST_TC_SPAN
    try:
        _LAST_TC_SPAN = max(e[2] for e in tc._perfetto_entries)
    except Exception:
        _LAST_TC_SPAN = 0
    nc.compile()
    return nc


_NC_CACHE = None
_LAST_IN_MAPS = None
_LAST_TC_SPAN = 0


def _get_nc():
    global _NC_CACHE
    if _NC_CACHE is None:
        _NC_CACHE = _build_bass()
    return _NC_CACHE


def kernel(**inputs):
    import sys
    if '/opt/trn_rl_repo' not in sys.path:
        sys.path.insert(0, '/opt/trn_rl_repo')
    from concourse.bass_utils import run_bass_kernel_spmd

    inp = {k: np.asarray(v) for k, v in inputs.items()}
    x = inp['x'].astype(np.float32)
    wq, wk, wv, wo = (inp[n].astype(np.float32) for n in ('wq_w', 'wk_w', 'wv_w', 'wo_w'))
    wg = inp['wg_w'].astype(np.float32).reshape(HD, HD)
    gbv = float(inp['wg_b'][0])
    kvs = inp['kv_norm_scale'].astype(np.float32)[0, :, 0]
    qks = inp['qk_norm_scale'].astype(np.float32)[0, :, 0]

    nc = _get_nc()

    bd2 = np.zeros((64, 2), np.float32)
    bd2[0:32, 0] = 1.0
    bd2[32:64, 1] = 1.0
    bd64 = np.zeros((2, 64), np.float32)
    bd64[0, 0:32] = 1.0
    bd64[1, 32:64] = 1.0
    lt32 = np.zeros((32, 32), np.float32)
    for p in range(32):
        for s in range(32):
            if p // 16 == s // 16 and s % 16 < p % 16:
                lt32[s, p] = 1.0
    ident = np.eye(128, dtype=np.float32)
    maskc = (np.arange(128)[:, None] <= np.arange(128)[None, :]).astype(np.float32)

    in_maps = []
    for core in range(N_CORES):
        b = core // 4
        heads = (2 * (core % 4), 2 * (core % 4) + 1)
        xT = np.ascontiguousarray(
            x[b].T.reshape(2, 128, L).transpose(1, 0, 2))       # [128,2,L]
        wqkvT = np.empty((D, 192), np.float32)
        for j, wmat in enumerate((wq, wk, wv)):
            for li, hh in enumerate(heads):
                wqkvT[:, j*64+li*32:j*64+(li+1)*32] = wmat[hh*HD:(hh+1)*HD, :].T
        wqkvT = np.ascontiguousarray(wqkvT.reshape(2, 128, 192).transpose(1, 0, 2))
        woT = np.empty((64, D), np.float32)
        for li, hh in enumerate(heads):
            woT[li*32:(li+1)*32, :] = wo[:, hh*HD:(hh+1)*HD].T
        bdwg = np.zeros((64, 64), np.float32)
        vecs = np.ones((64, 2), np.float32)
        for li, hh in enumerate(heads):
            sc = kvs[hh]
            a = sc[:, 0].copy()
            bvec = sc[0, :] / sc[0, 0]
            Wg_h = wg * sc / np.outer(a, bvec)
            bdwg[li*32:(li+1)*32, li*32:(li+1)*32] = Wg_h.T
            vecs[li*32:(li+1)*32, 0] = a
            vecs[li*32:(li+1)*32, 1] = bvec
        qs32 = np.empty((32, 1), np.float32)
        qs32[0:16] = qks[heads[0]]
        qs32[16:32] = qks[heads[1]]
        gb32 = np.full((32, 1), gbv, np.float32)
        in_maps.append({
            "xT": xT, "wqkvT": wqkvT, "woT": woT, "bdwg": bdwg,
            "bd2": bd2, "bd64": bd64, "lt32": lt32, "ident": ident,
            "mask": maskc, "qs32": qs32, "vecs": vecs, "gb": gb32,
        })

    global _LAST_IN_MAPS
    _LAST_IN_MAPS = in_maps
    res = run_bass_kernel_spmd(nc, in_maps, core_ids=list(range(N_CORES)))
    out = np.zeros((B, L, D), np.float32)
    for core in range(N_CORES):
        out[core // 4] += res.results[core]["y"]
    out += inp['wo_b'].astype(np.float32)[None, None, :]
    return out
